# revision 17
# baseline (speedup 1.0000x reference)
"""HarmonyGenerator Trainium2 kernel.

Math: the reference's 3x3 conv on [T,1,1,D] degenerates to a 3-tap conv along
the feature axis (only the kernel's middle row touches data).  Conv and the
three linear heads are both linear, so the conv folds into the head weights
(W' = 3-tap correlation of W along K) and the constant context-embedding rows
plus conv bias fold into the output bias.  The device work is one GEMM:

    out[2048, 168] = [melody | lyrics][2048, 50681] @ W'[50681, 168] + bias

Sharding: K (feature) axis split 8 ways, 6400 rows per core (zero padded).
Each core reads 1/8 of x AND 1/8 of W and produces a partial [168, 2048]
(fp16); partials are summed on the host during the gather/unshard step.

Schedule (v2): x and W both stream on the two HWDGE rings (sync + scalar) --
the SWDGE ring moves W at only ~7 B/ns/queue and drags the hardware queues
down with it.  W chunks are interleaved a few positions ahead of the x
chunks that need them; a greedy byte-balancer assigns chunks to rings.
Head processing is batched (4-8 k-tiles of 128-col mel matmuls, then the
40-col chord+beat pairs) because switching the PE column-group config
costs ~95 ns while a weight reload inside one config hides under the
previous matmul's stream.  The tail keeps the last chunks small and
interleaves the kt-49 matmuls with PSUM evictions so only ~2 us of work
remains after the last x byte lands.
"""

import os
import numpy as np

import concourse.bacc as bacc
import concourse.mybir as mybir
from concourse.tile import TileContext
from concourse.bass_utils import run_bass_kernel_spmd

# Problem shapes (hardcoded per contract)
T = 2048               # steps = length * 128
D_IN = 50937           # 256 ctx + 256 melody/vel + 50425 lyrics
K_GEMM = 50681         # melody(256) + lyrics(50425) features in the GEMM
N_OUT = 168            # 24 chord + 16 beat + 128 mel
N_CORES = 8
K_PER = 6400           # per-core K (8*6400 = 51200 >= 50681, zero padded)
KT = K_PER // 128      # 50 k-tiles per core
TB = 512               # t-block (max fp32 moving dim / PSUM bank)
NTB = T // TB          # 4
KT_LAST = KT - 1

# x chunks (start_kt, n_kt): small in the ramp phase (the PE runs at DMA
# speed, so it waits for whole chunks -- keep the quantum fine), 2 MB in the
# body, small tail so little work trails the last byte.
X_CHUNKS = [(0, 1), (1, 1), (2, 2), (4, 2), (6, 2), (8, 2),
            (10, 4), (14, 4), (18, 4), (22, 4), (26, 4), (30, 4),
            (34, 4), (38, 4), (42, 4), (46, 2), (48, 1), (49, 1)]
W_CHUNKS = [(0, 4), (4, 6), (10, 8), (18, 8), (26, 8), (34, 8), (42, 8)]
# issue order: each W chunk lands a few positions before the x it gates.
# A gated DMA blocks everything behind it on its ring, so chunk DMAs must
# never be issued closer than the buffer pool depth allows.
ORDER = [('x', 0), ('w', 0), ('x', 1), ('w', 1), ('x', 2), ('x', 3),
         ('w', 2), ('x', 4), ('x', 5), ('w', 3), ('x', 6), ('x', 7),
         ('w', 4), ('x', 8), ('x', 9), ('w', 5), ('x', 10), ('x', 11),
         ('w', 6), ('x', 12), ('x', 13), ('x', 14), ('x', 15), ('x', 16),
         ('x', 17)]
# PE batches (start_kt, n_kt): mel block then cb block per batch
BATCHES = [(0, 4), (4, 6), (10, 8), (18, 8), (26, 8), (34, 8), (42, 4), (46, 4)]

assert sum(n for _, n in X_CHUNKS) == KT
assert sum(n for _, n in W_CHUNKS) == KT
assert sum(n for _, n in BATCHES) == KT

_NC = None
LAST_RESULT = None     # BassKernelResults of the most recent run (for test.py)


def _build_nc():
    f32 = mybir.dt.float32
    f16 = mybir.dt.float16
    nc = bacc.Bacc()
    xt = nc.dram_tensor("xt", [K_PER, T], f16, kind="ExternalInput")
    w = nc.dram_tensor("w", [128, KT * N_OUT], f16, kind="ExternalInput")
    out = nc.dram_tensor("out", [N_OUT, T], f16, kind="ExternalOutput")

    with TileContext(nc) as tc:
        with (
            tc.tile_pool(name="wp", bufs=1) as wp,
            tc.tile_pool(name="xp", bufs=1) as xp,
            tc.tile_pool(name="op", bufs=2) as op,
            tc.tile_pool(name="ps", bufs=1, space="PSUM") as ps,
        ):
            # HAM warm-up: the PE clock-gate holds matmuls at low clock until
            # ~3.4us of sustained activity.  Burn the DMA-fill window on dummy
            # matmuls (ending in the full 128-col config) so real MMs start
            # fast.  Scratch PSUM bank; results never read.
            dm = wp.tile([128, TB], f16, tag="warm", name="warmup")
            nc.gpsimd.memset(dm[:], 0.0)
            ps_warm = ps.tile([128, TB], f32, tag="warm_ps", name="ps_warm")
            for _ in range(8):
                nc.tensor.matmul(ps_warm[:], dm[:, 0:128], dm[:], start=True, stop=True)

            # Build W tiles and x tiles; issue DMAs on the two HWDGE rings,
            # greedily balancing queued bytes per ring.
            w_of = {}
            x_of = {}
            w_tiles = {}
            x_tiles = {}
            ring_bytes = [0, 0]
            rings = [nc.sync, nc.scalar]
            for kind, idx in ORDER:
                if kind == 'w':
                    s, n = W_CHUNKS[idx]
                    tile = wp.tile([128, n * N_OUT], f16, tag=f"w{idx}", name=f"w{idx}")
                    nbytes = 128 * n * N_OUT * 2
                    r = 0 if ring_bytes[0] <= ring_bytes[1] else 1
                    ring_bytes[r] += nbytes
                    rings[r].dma_start(tile[:], w[:, s * N_OUT:(s + n) * N_OUT])
                    for j in range(n):
                        w_of[s + j] = (tile, j * N_OUT)
                    w_tiles[idx] = tile
                else:
                    s, n = X_CHUNKS[idx]
                    tile = xp.tile([128, n * T], f16, tag=f"x{n}", name=f"x{n}_{idx}",
                                   bufs={1: 2, 2: 3, 4: 5}[n])
                    nbytes = 128 * n * T * 2
                    r = 0 if ring_bytes[0] <= ring_bytes[1] else 1
                    ring_bytes[r] += nbytes
                    if n == 1:
                        rings[r].dma_start(tile[:], xt[s * 128:(s + 1) * 128, :])
                    else:
                        rings[r].dma_start(
                            tile[:].rearrange("p (a t) -> p a t", a=n),
                            xt[s * 128:(s + n) * 128, :].rearrange(
                                "(a p) t -> p a t", p=128),
                        )
                    for j in range(n):
                        x_of[s + j] = (tile, j * T)

            # Persistent accumulators: 4 mel banks + 2 shared cb banks.  Each
            # cb bank holds two t-blocks' [40, TB] outputs col-tiled into
            # partitions 0:40 and 64:104 (concurrent matmuls via tile_position).
            psm = [ps.tile([128, TB], f32, tag=f"m{t}", name=f"psm{t}") for t in range(NTB)]
            psc = [ps.tile([128, TB], f32, tag=f"c{p}", name=f"psc{p}") for p in range(NTB // 2)]

            def rhs_of(kt, t):
                tile, off = x_of[kt]
                return tile[:, off + t * TB: off + (t + 1) * TB]

            def mel(kt, t):
                wt, j = w_of[kt]
                nc.tensor.matmul(psm[t][:], wt[:, j: j + 128], rhs_of(kt, t),
                                 start=(kt == 0), stop=(kt == KT_LAST))

            def cb_pair(kt, p):
                wt, j = w_of[kt]
                lhs_c = wt[:, j + 128: j + N_OUT]
                nc.tensor.matmul(psc[p][0:40, :], lhs_c, rhs_of(kt, 2 * p),
                                 start=(kt == 0), stop=(kt == KT_LAST),
                                 tile_position=(0, 0))
                nc.tensor.matmul(psc[p][64:104, :], lhs_c, rhs_of(kt, 2 * p + 1),
                                 start=(kt == 0), stop=(kt == KT_LAST),
                                 tile_position=(0, 64))

            def evict_mel(t):
                o = op.tile([128, TB], f16, tag=f"om{t % 2}", name=f"om{t}")
                if t % 2 == 0:
                    nc.vector.tensor_copy(o[:], psm[t][:])
                    nc.sync.dma_start(out[0:128, t * TB:(t + 1) * TB], o[:])
                else:
                    nc.scalar.copy(o[:], psm[t][:])
                    nc.scalar.dma_start(out[0:128, t * TB:(t + 1) * TB], o[:])

            def evict_cb(p):
                o = op.tile([104, TB], f16, tag="oc", name=f"oc{p}")
                if p == 0:
                    nc.vector.tensor_copy(o[0:104, :], psc[p][0:104, :])
                    ring = nc.sync
                else:
                    nc.scalar.copy(o[0:104, :], psc[p][0:104, :])
                    ring = nc.scalar
                ring.dma_start(out[128:N_OUT, 2 * p * TB:(2 * p + 1) * TB], o[0:40, :])
                ring.dma_start(out[128:N_OUT, (2 * p + 1) * TB:(2 * p + 2) * TB], o[64:104, :])

            # The cb pairs of chunk c run right before the mels of chunk
            # c+1: pairs touch only already-resident data, so when the mels
            # would stall on a fresh chunk arrival the PE fills the wait
            # with pair work instead of idling (the PE is in-order, so
            # pairs placed after stalling mels could never slide forward).
            for ci, (s, n) in enumerate(X_CHUNKS):
                if ci > 0:
                    ps_, pn = X_CHUNKS[ci - 1]
                    for kt in range(ps_, ps_ + pn):
                        cb_pair(kt, 0)
                        cb_pair(kt, 1)
                if ci < len(X_CHUNKS) - 1:
                    for kt in range(s, s + n):
                        for t in range(NTB):
                            mel(kt, t)
                else:
                    # kt49: bank-by-bank with evictions overlapping the
                    # remaining matmuls
                    for t in range(NTB):
                        mel(KT_LAST, t)
                        evict_mel(t)
            cb_pair(KT_LAST, 0)
            evict_cb(0)
            cb_pair(KT_LAST, 1)
            evict_cb(1)
    return nc


def _get_nc():
    global _NC
    if _NC is None:
        _NC = _build_nc()
        if not _NC.is_finalized():
            _NC.finalize()
    return _NC


def kernel(**inputs):
    global LAST_RESULT
    melody = np.ascontiguousarray(np.asarray(inputs["melody_tensor"], dtype=np.float32))
    lyrics = np.ascontiguousarray(np.asarray(inputs["lyrics_tensor"], dtype=np.float32))
    emb = np.asarray(inputs["emb"], dtype=np.float32)
    conv_w = np.asarray(inputs["conv_w"], dtype=np.float32)
    conv_b = np.asarray(inputs["conv_b"], dtype=np.float32)
    w_chord = np.asarray(inputs["w_chord"], dtype=np.float32)
    w_beat = np.asarray(inputs["w_beat"], dtype=np.float32)
    w_mel = np.asarray(inputs["w_mel"], dtype=np.float32)
    b_heads = np.concatenate([
        np.asarray(inputs["b_chord"], dtype=np.float32),
        np.asarray(inputs["b_beat"], dtype=np.float32),
        np.asarray(inputs["b_mel"], dtype=np.float32),
    ])
    genre = int(np.asarray(inputs["genre"]).reshape(-1)[0])
    tempo = int(np.asarray(inputs["tempo"]).reshape(-1)[0])
    key_sig = int(np.asarray(inputs["key_sig"]).reshape(-1)[0])

    # Fold conv into head weights: W'[e] = k0*W[e+1] + k1*W[e] + k2*W[e-1]
    W = np.concatenate([w_chord, w_beat, w_mel], axis=1)  # [50937, 168]
    k0, k1, k2 = (float(v) for v in conv_w[0, 0, 1, :])
    Wp = k1 * W
    Wp[:-1] += k0 * W[1:]
    Wp[1:] += k2 * W[:-1]

    # Bias: head biases + conv bias * colsum(W) + context-embedding term
    ids = [genre, 10 + tempo, 20 + key_sig, 34]
    ctx = emb[ids].sum(axis=0).astype(np.float64)  # [256]
    bias = (
        b_heads.astype(np.float64)
        + float(conv_b[0]) * W.sum(axis=0, dtype=np.float64)
        + ctx @ Wp[0:256].astype(np.float64)
    )  # [168]

    # Device operands: xT [51200, 2048] (zero padded), W' rows 256.. packed
    # [128, kt*168] with head weights per k-tile: [mel 128 | chord+beat 40]
    K_PAD = N_CORES * K_PER
    XT = np.zeros((K_PAD, T), np.float16)
    XT[0:256] = melody.T
    XT[256:K_GEMM] = lyrics.T
    Wg = np.zeros((K_PAD, N_OUT), np.float16)
    Wg[0:K_GEMM] = Wp[256:]

    in_maps = []
    for c in range(N_CORES):
        wc = (
            Wg[c * K_PER:(c + 1) * K_PER]
            .reshape(KT, 128, N_OUT)
            .transpose(1, 0, 2)
            .reshape(128, KT * N_OUT)
        )
        in_maps.append({
            "xt": XT[c * K_PER:(c + 1) * K_PER],
            "w": np.ascontiguousarray(wc),
        })

    trace = bool(os.environ.get("HARMONY_TRACE"))
    res = run_bass_kernel_spmd(_get_nc(), in_maps, core_ids=list(range(N_CORES)), trace=trace)
    LAST_RESULT = res

    acc = np.zeros((N_OUT, T), np.float64)
    for r in res.results:
        acc += r["out"].astype(np.float64)
    out = (acc + bias[:, None]).T
    return np.ascontiguousarray(out.astype(np.float32))


# revision 18
# speedup vs baseline: 1.1267x; 1.1267x over previous
"""HarmonyGenerator Trainium2 kernel.

Math: the reference's 3x3 conv on [T,1,1,D] degenerates to a 3-tap conv along
the feature axis (only the kernel's middle row touches data).  Conv and the
three linear heads are both linear, so the conv folds into the head weights
(W' = 3-tap correlation of W along K) and the constant context-embedding rows
plus conv bias fold into the output bias.  The device work is one GEMM:

    out[2048, 168] = [melody | lyrics][2048, 50681] @ W'[50681, 168] + bias

Sharding: K (feature) axis split 8 ways, 6400 rows per core (zero padded).
Each core reads 1/8 of x AND 1/8 of W and produces a partial [168, 2048]
(fp16); partials are summed on the host during the gather/unshard step.

Schedule (v2): x and W both stream on the two HWDGE rings (sync + scalar) --
the SWDGE ring moves W at only ~7 B/ns/queue and drags the hardware queues
down with it.  W chunks are interleaved a few positions ahead of the x
chunks that need them; a greedy byte-balancer assigns chunks to rings.
Head processing is batched (4-8 k-tiles of 128-col mel matmuls, then the
40-col chord+beat pairs) because switching the PE column-group config
costs ~95 ns while a weight reload inside one config hides under the
previous matmul's stream.  The tail keeps the last chunks small and
interleaves the kt-49 matmuls with PSUM evictions so only ~2 us of work
remains after the last x byte lands.
"""

import os
import numpy as np

import concourse.bacc as bacc
import concourse.mybir as mybir
from concourse.tile import TileContext
from concourse.bass_utils import run_bass_kernel_spmd

# Problem shapes (hardcoded per contract)
T = 2048               # steps = length * 128
D_IN = 50937           # 256 ctx + 256 melody/vel + 50425 lyrics
K_GEMM = 50681         # melody(256) + lyrics(50425) features in the GEMM
N_OUT = 168            # 24 chord + 16 beat + 128 mel
N_CORES = 8
K_PER = 6400           # per-core K (8*6400 = 51200 >= 50681, zero padded)
KT = K_PER // 128      # 50 k-tiles per core
TB = 512               # t-block (max fp32 moving dim / PSUM bank)
NTB = T // TB          # 4
KT_LAST = KT - 1

# x chunks (start_kt, n_kt): small in the ramp phase (the PE runs at DMA
# speed, so it waits for whole chunks -- keep the quantum fine), 2 MB in the
# body, small tail so little work trails the last byte.
X_CHUNKS = [(0, 1), (1, 1), (2, 2), (4, 2), (6, 2), (8, 2),
            (10, 4), (14, 4), (18, 4), (22, 4), (26, 4), (30, 4),
            (34, 4), (38, 4), (42, 4), (46, 2), (48, 1), (49, 1)]
W_CHUNKS = [(0, 4), (4, 6), (10, 8), (18, 8), (26, 8), (34, 8), (42, 8)]
# issue order: each W chunk lands a few positions before the x it gates.
# A gated DMA blocks everything behind it on its ring, so chunk DMAs must
# never be issued closer than the buffer pool depth allows.
ORDER = [('x', 0), ('w', 0), ('x', 1), ('w', 1), ('x', 2), ('x', 3),
         ('w', 2), ('x', 4), ('x', 5), ('w', 3), ('x', 6), ('x', 7),
         ('w', 4), ('x', 8), ('x', 9), ('w', 5), ('x', 10), ('x', 11),
         ('w', 6), ('x', 12), ('x', 13), ('x', 14), ('x', 15), ('x', 16),
         ('x', 17)]
# PE batches (start_kt, n_kt): mel block then cb block per batch
BATCHES = [(0, 4), (4, 6), (10, 8), (18, 8), (26, 8), (34, 8), (42, 4), (46, 4)]

assert sum(n for _, n in X_CHUNKS) == KT
assert sum(n for _, n in W_CHUNKS) == KT
assert sum(n for _, n in BATCHES) == KT

_NC = None
LAST_RESULT = None     # BassKernelResults of the most recent run (for test.py)


def _build_nc():
    f32 = mybir.dt.float32
    f16 = mybir.dt.float16
    nc = bacc.Bacc()
    xt = nc.dram_tensor("xt", [K_PER, T], f16, kind="ExternalInput")
    w = nc.dram_tensor("w", [128, KT * N_OUT], f16, kind="ExternalInput")
    out = nc.dram_tensor("out", [N_OUT, T], f16, kind="ExternalOutput")

    with TileContext(nc) as tc:
        with (
            tc.tile_pool(name="wp", bufs=1) as wp,
            tc.tile_pool(name="xp", bufs=1) as xp,
            tc.tile_pool(name="op", bufs=2) as op,
            tc.tile_pool(name="ps", bufs=1, space="PSUM") as ps,
        ):
            # HAM warm-up: the PE clock-gate holds matmuls at low clock until
            # ~3.4us of sustained activity.  Burn the DMA-fill window on dummy
            # matmuls (ending in the full 128-col config) so real MMs start
            # fast.  Scratch PSUM bank; results never read.
            dm = wp.tile([128, TB], f16, tag="warm", name="warmup")
            nc.gpsimd.memset(dm[:], 0.0)
            ps_warm = ps.tile([128, TB], f32, tag="warm_ps", name="ps_warm")
            for _ in range(8):
                nc.tensor.matmul(ps_warm[:], dm[:, 0:128], dm[:], start=True, stop=True)

            # Build W tiles and x tiles; issue DMAs on the two HWDGE rings,
            # greedily balancing queued bytes per ring.
            w_of = {}
            x_of = {}
            w_tiles = {}
            x_tiles = {}
            ring_bytes = [0, 0]
            rings = [nc.sync, nc.scalar]
            for kind, idx in ORDER:
                if kind == 'w':
                    s, n = W_CHUNKS[idx]
                    tile = wp.tile([128, n * N_OUT], f16, tag=f"w{idx}", name=f"w{idx}")
                    nbytes = 128 * n * N_OUT * 2
                    r = 0 if ring_bytes[0] <= ring_bytes[1] else 1
                    ring_bytes[r] += nbytes
                    rings[r].dma_start(tile[:], w[:, s * N_OUT:(s + n) * N_OUT])
                    for j in range(n):
                        w_of[s + j] = (tile, j * N_OUT)
                    w_tiles[idx] = tile
                else:
                    s, n = X_CHUNKS[idx]
                    tile = xp.tile([128, n * T], f16, tag=f"x{n}", name=f"x{n}_{idx}",
                                   bufs={1: 2, 2: 4, 4: 7}[n])
                    nbytes = 128 * n * T * 2
                    r = 0 if ring_bytes[0] <= ring_bytes[1] else 1
                    ring_bytes[r] += nbytes
                    if n == 1:
                        rings[r].dma_start(tile[:], xt[s * 128:(s + 1) * 128, :])
                    else:
                        rings[r].dma_start(
                            tile[:].rearrange("p (a t) -> p a t", a=n),
                            xt[s * 128:(s + n) * 128, :].rearrange(
                                "(a p) t -> p a t", p=128),
                        )
                    for j in range(n):
                        x_of[s + j] = (tile, j * T)

            # Persistent accumulators: 4 mel banks + 2 shared cb banks.  Each
            # cb bank holds two t-blocks' [40, TB] outputs col-tiled into
            # partitions 0:40 and 64:104 (concurrent matmuls via tile_position).
            psm = [ps.tile([128, TB], f32, tag=f"m{t}", name=f"psm{t}") for t in range(NTB)]
            psc = [ps.tile([128, TB], f32, tag=f"c{p}", name=f"psc{p}") for p in range(NTB // 2)]

            def rhs_of(kt, t):
                tile, off = x_of[kt]
                return tile[:, off + t * TB: off + (t + 1) * TB]

            def mel(kt, t):
                wt, j = w_of[kt]
                nc.tensor.matmul(psm[t][:], wt[:, j: j + 128], rhs_of(kt, t),
                                 start=(kt == 0), stop=(kt == KT_LAST))

            def cb_pair(kt, p):
                wt, j = w_of[kt]
                lhs_c = wt[:, j + 128: j + N_OUT]
                nc.tensor.matmul(psc[p][0:40, :], lhs_c, rhs_of(kt, 2 * p),
                                 start=(kt == 0), stop=(kt == KT_LAST),
                                 tile_position=(0, 0))
                nc.tensor.matmul(psc[p][64:104, :], lhs_c, rhs_of(kt, 2 * p + 1),
                                 start=(kt == 0), stop=(kt == KT_LAST),
                                 tile_position=(0, 64))

            def evict_mel(t):
                o = op.tile([128, TB], f16, tag=f"om{t % 2}", name=f"om{t}")
                if t % 2 == 0:
                    nc.vector.tensor_copy(o[:], psm[t][:])
                    nc.sync.dma_start(out[0:128, t * TB:(t + 1) * TB], o[:])
                else:
                    nc.scalar.copy(o[:], psm[t][:])
                    nc.scalar.dma_start(out[0:128, t * TB:(t + 1) * TB], o[:])

            def evict_cb(p):
                o = op.tile([104, TB], f16, tag="oc", name=f"oc{p}")
                if p == 0:
                    nc.vector.tensor_copy(o[0:104, :], psc[p][0:104, :])
                    ring = nc.sync
                else:
                    nc.scalar.copy(o[0:104, :], psc[p][0:104, :])
                    ring = nc.scalar
                ring.dma_start(out[128:N_OUT, 2 * p * TB:(2 * p + 1) * TB], o[0:40, :])
                ring.dma_start(out[128:N_OUT, (2 * p + 1) * TB:(2 * p + 2) * TB], o[64:104, :])

            # The cb pairs of chunk c run right before the mels of chunk
            # c+1: pairs touch only already-resident data, so when the mels
            # would stall on a fresh chunk arrival the PE fills the wait
            # with pair work instead of idling (the PE is in-order, so
            # pairs placed after stalling mels could never slide forward).
            for ci, (s, n) in enumerate(X_CHUNKS):
                if ci > 0:
                    ps_, pn = X_CHUNKS[ci - 1]
                    for kt in range(ps_, ps_ + pn):
                        cb_pair(kt, 0)
                        cb_pair(kt, 1)
                if ci < len(X_CHUNKS) - 1:
                    for kt in range(s, s + n):
                        for t in range(NTB):
                            mel(kt, t)
                else:
                    # kt49: bank-by-bank with evictions overlapping the
                    # remaining matmuls
                    for t in range(NTB):
                        mel(KT_LAST, t)
                        evict_mel(t)
            cb_pair(KT_LAST, 0)
            evict_cb(0)
            cb_pair(KT_LAST, 1)
            evict_cb(1)
    return nc


def _get_nc():
    global _NC
    if _NC is None:
        _NC = _build_nc()
        if not _NC.is_finalized():
            _NC.finalize()
    return _NC


def kernel(**inputs):
    global LAST_RESULT
    melody = np.ascontiguousarray(np.asarray(inputs["melody_tensor"], dtype=np.float32))
    lyrics = np.ascontiguousarray(np.asarray(inputs["lyrics_tensor"], dtype=np.float32))
    emb = np.asarray(inputs["emb"], dtype=np.float32)
    conv_w = np.asarray(inputs["conv_w"], dtype=np.float32)
    conv_b = np.asarray(inputs["conv_b"], dtype=np.float32)
    w_chord = np.asarray(inputs["w_chord"], dtype=np.float32)
    w_beat = np.asarray(inputs["w_beat"], dtype=np.float32)
    w_mel = np.asarray(inputs["w_mel"], dtype=np.float32)
    b_heads = np.concatenate([
        np.asarray(inputs["b_chord"], dtype=np.float32),
        np.asarray(inputs["b_beat"], dtype=np.float32),
        np.asarray(inputs["b_mel"], dtype=np.float32),
    ])
    genre = int(np.asarray(inputs["genre"]).reshape(-1)[0])
    tempo = int(np.asarray(inputs["tempo"]).reshape(-1)[0])
    key_sig = int(np.asarray(inputs["key_sig"]).reshape(-1)[0])

    # Fold conv into head weights: W'[e] = k0*W[e+1] + k1*W[e] + k2*W[e-1]
    W = np.concatenate([w_chord, w_beat, w_mel], axis=1)  # [50937, 168]
    k0, k1, k2 = (float(v) for v in conv_w[0, 0, 1, :])
    Wp = k1 * W
    Wp[:-1] += k0 * W[1:]
    Wp[1:] += k2 * W[:-1]

    # Bias: head biases + conv bias * colsum(W) + context-embedding term
    ids = [genre, 10 + tempo, 20 + key_sig, 34]
    ctx = emb[ids].sum(axis=0).astype(np.float64)  # [256]
    bias = (
        b_heads.astype(np.float64)
        + float(conv_b[0]) * W.sum(axis=0, dtype=np.float64)
        + ctx @ Wp[0:256].astype(np.float64)
    )  # [168]

    # Device operands: xT [51200, 2048] (zero padded), W' rows 256.. packed
    # [128, kt*168] with head weights per k-tile: [mel 128 | chord+beat 40]
    K_PAD = N_CORES * K_PER
    XT = np.zeros((K_PAD, T), np.float16)
    XT[0:256] = melody.T
    XT[256:K_GEMM] = lyrics.T
    Wg = np.zeros((K_PAD, N_OUT), np.float16)
    Wg[0:K_GEMM] = Wp[256:]

    in_maps = []
    for c in range(N_CORES):
        wc = (
            Wg[c * K_PER:(c + 1) * K_PER]
            .reshape(KT, 128, N_OUT)
            .transpose(1, 0, 2)
            .reshape(128, KT * N_OUT)
        )
        in_maps.append({
            "xt": XT[c * K_PER:(c + 1) * K_PER],
            "w": np.ascontiguousarray(wc),
        })

    trace = bool(os.environ.get("HARMONY_TRACE"))
    res = run_bass_kernel_spmd(_get_nc(), in_maps, core_ids=list(range(N_CORES)), trace=trace)
    LAST_RESULT = res

    acc = np.zeros((N_OUT, T), np.float64)
    for r in res.results:
        acc += r["out"].astype(np.float64)
    out = (acc + bias[:, None]).T
    return np.ascontiguousarray(out.astype(np.float32))


# revision 19
# speedup vs baseline: 1.1860x; 1.0526x over previous
"""HarmonyGenerator Trainium2 kernel.

Math: the reference's 3x3 conv on [T,1,1,D] degenerates to a 3-tap conv along
the feature axis (only the kernel's middle row touches data).  Conv and the
three linear heads are both linear, so the conv folds into the head weights
(W' = 3-tap correlation of W along K) and the constant context-embedding rows
plus conv bias fold into the output bias.  The device work is one GEMM:

    out[2048, 168] = [melody | lyrics][2048, 50681] @ W'[50681, 168] + bias

Sharding: K (feature) axis split 8 ways, 6400 rows per core (zero padded).
Each core reads 1/8 of x AND 1/8 of W and produces a partial [168, 2048]
(fp16); partials are summed on the host during the gather/unshard step.

Schedule (v2): x and W both stream on the two HWDGE rings (sync + scalar) --
the SWDGE ring moves W at only ~7 B/ns/queue and drags the hardware queues
down with it.  W chunks are interleaved a few positions ahead of the x
chunks that need them; a greedy byte-balancer assigns chunks to rings.
Head processing is batched (4-8 k-tiles of 128-col mel matmuls, then the
40-col chord+beat pairs) because switching the PE column-group config
costs ~95 ns while a weight reload inside one config hides under the
previous matmul's stream.  The tail keeps the last chunks small and
interleaves the kt-49 matmuls with PSUM evictions so only ~2 us of work
remains after the last x byte lands.
"""

import os
import numpy as np

import concourse.bacc as bacc
import concourse.mybir as mybir
from concourse.tile import TileContext
from concourse.bass_utils import run_bass_kernel_spmd

# Problem shapes (hardcoded per contract)
T = 2048               # steps = length * 128
D_IN = 50937           # 256 ctx + 256 melody/vel + 50425 lyrics
K_GEMM = 50681         # melody(256) + lyrics(50425) features in the GEMM
N_OUT = 168            # 24 chord + 16 beat + 128 mel
N_CORES = 8
K_PER = 6400           # per-core K (8*6400 = 51200 >= 50681, zero padded)
KT = K_PER // 128      # 50 k-tiles per core
TB = 512               # t-block (max fp32 moving dim / PSUM bank)
NTB = T // TB          # 4
KT_LAST = KT - 1

# x chunks (start_kt, n_kt): small in the ramp phase (the PE runs at DMA
# speed, so it waits for whole chunks -- keep the quantum fine), 2 MB in the
# body, small tail so little work trails the last byte.
X_CHUNKS = [(0, 1), (1, 1), (2, 2), (4, 2), (6, 2), (8, 2),
            (10, 4), (14, 4), (18, 4), (22, 4), (26, 4), (30, 4),
            (34, 4), (38, 4), (42, 4), (46, 2), (48, 1), (49, 1)]
W_CHUNKS = [(0, 4), (4, 6), (10, 8), (18, 8), (26, 8), (34, 8), (42, 8)]
# issue order: each W chunk lands a few positions before the x it gates.
# A gated DMA blocks everything behind it on its ring, so chunk DMAs must
# never be issued closer than the buffer pool depth allows.
ORDER = [('x', 0), ('w', 0), ('x', 1), ('w', 1), ('x', 2), ('x', 3),
         ('w', 2), ('x', 4), ('x', 5), ('w', 3), ('x', 6), ('x', 7),
         ('w', 4), ('x', 8), ('x', 9), ('w', 5), ('x', 10), ('x', 11),
         ('w', 6), ('x', 12), ('x', 13), ('x', 14), ('x', 15), ('x', 16),
         ('x', 17)]
# PE batches (start_kt, n_kt): mel block then cb block per batch
BATCHES = [(0, 4), (4, 6), (10, 8), (18, 8), (26, 8), (34, 8), (42, 4), (46, 4)]

assert sum(n for _, n in X_CHUNKS) == KT
assert sum(n for _, n in W_CHUNKS) == KT
assert sum(n for _, n in BATCHES) == KT

_NC = None
LAST_RESULT = None     # BassKernelResults of the most recent run (for test.py)


def _build_nc():
    f32 = mybir.dt.float32
    f16 = mybir.dt.float16
    nc = bacc.Bacc()
    xt = nc.dram_tensor("xt", [K_PER, T], f16, kind="ExternalInput")
    w = nc.dram_tensor("w", [128, KT * N_OUT], f16, kind="ExternalInput")
    out = nc.dram_tensor("out", [N_OUT, T], f16, kind="ExternalOutput")

    with TileContext(nc) as tc:
        with (
            tc.tile_pool(name="wp", bufs=1) as wp,
            tc.tile_pool(name="xp", bufs=1) as xp,
            tc.tile_pool(name="op", bufs=2) as op,
            tc.tile_pool(name="ps", bufs=1, space="PSUM") as ps,
        ):
            # HAM warm-up: the PE clock-gate holds matmuls at low clock until
            # ~3.4us of sustained activity.  Burn the DMA-fill window on dummy
            # matmuls (ending in the full 128-col config) so real MMs start
            # fast.  Scratch PSUM bank; results never read.
            dm = wp.tile([128, TB], f16, tag="warm", name="warmup")
            nc.gpsimd.memset(dm[:], 0.0)
            ps_warm = ps.tile([128, TB], f32, tag="warm_ps", name="ps_warm")
            for _ in range(8):
                nc.tensor.matmul(ps_warm[:], dm[:, 0:128], dm[:], start=True, stop=True)

            # Build W tiles and x tiles; issue DMAs on the two HWDGE rings,
            # greedily balancing queued bytes per ring.
            w_of = {}
            x_of = {}
            w_tiles = {}
            x_tiles = {}
            ring_bytes = [0, 0]
            rings = [nc.sync, nc.scalar]
            for kind, idx in ORDER:
                if kind == 'w':
                    s, n = W_CHUNKS[idx]
                    tile = wp.tile([128, n * N_OUT], f16, tag=f"w{idx}", name=f"w{idx}")
                    nbytes = 128 * n * N_OUT * 2
                    r = 0 if ring_bytes[0] <= ring_bytes[1] else 1
                    ring_bytes[r] += nbytes
                    rings[r].dma_start(tile[:], w[:, s * N_OUT:(s + n) * N_OUT])
                    for j in range(n):
                        w_of[s + j] = (tile, j * N_OUT)
                    w_tiles[idx] = tile
                else:
                    s, n = X_CHUNKS[idx]
                    tile = xp.tile([128, n * T], f16, tag=f"x{n}", name=f"x{n}_{idx}",
                                   bufs={1: 2, 2: 4, 4: 8}[n])
                    nbytes = 128 * n * T * 2
                    r = 0 if ring_bytes[0] <= ring_bytes[1] else 1
                    ring_bytes[r] += nbytes
                    if n == 1:
                        rings[r].dma_start(tile[:], xt[s * 128:(s + 1) * 128, :])
                    else:
                        rings[r].dma_start(
                            tile[:].rearrange("p (a t) -> p a t", a=n),
                            xt[s * 128:(s + n) * 128, :].rearrange(
                                "(a p) t -> p a t", p=128),
                        )
                    for j in range(n):
                        x_of[s + j] = (tile, j * T)

            # Persistent accumulators: 4 mel banks + 2 shared cb banks.  Each
            # cb bank holds two t-blocks' [40, TB] outputs col-tiled into
            # partitions 0:40 and 64:104 (concurrent matmuls via tile_position).
            psm = [ps.tile([128, TB], f32, tag=f"m{t}", name=f"psm{t}") for t in range(NTB)]
            psc = [ps.tile([128, TB], f32, tag=f"c{p}", name=f"psc{p}") for p in range(NTB // 2)]

            def rhs_of(kt, t):
                tile, off = x_of[kt]
                return tile[:, off + t * TB: off + (t + 1) * TB]

            def mel(kt, t):
                wt, j = w_of[kt]
                nc.tensor.matmul(psm[t][:], wt[:, j: j + 128], rhs_of(kt, t),
                                 start=(kt == 0), stop=(kt == KT_LAST))

            def cb_pair(kt, p):
                wt, j = w_of[kt]
                lhs_c = wt[:, j + 128: j + N_OUT]
                nc.tensor.matmul(psc[p][0:40, :], lhs_c, rhs_of(kt, 2 * p),
                                 start=(kt == 0), stop=(kt == KT_LAST),
                                 tile_position=(0, 0))
                nc.tensor.matmul(psc[p][64:104, :], lhs_c, rhs_of(kt, 2 * p + 1),
                                 start=(kt == 0), stop=(kt == KT_LAST),
                                 tile_position=(0, 64))

            def evict_mel(t):
                o = op.tile([128, TB], f16, tag=f"om{t % 2}", name=f"om{t}")
                if t % 2 == 0:
                    nc.vector.tensor_copy(o[:], psm[t][:])
                    nc.sync.dma_start(out[0:128, t * TB:(t + 1) * TB], o[:])
                else:
                    nc.scalar.copy(o[:], psm[t][:])
                    nc.scalar.dma_start(out[0:128, t * TB:(t + 1) * TB], o[:])

            def evict_cb(p):
                o = op.tile([104, TB], f16, tag="oc", name=f"oc{p}")
                if p == 0:
                    nc.vector.tensor_copy(o[0:104, :], psc[p][0:104, :])
                    ring = nc.sync
                else:
                    nc.scalar.copy(o[0:104, :], psc[p][0:104, :])
                    ring = nc.scalar
                ring.dma_start(out[128:N_OUT, 2 * p * TB:(2 * p + 1) * TB], o[0:40, :])
                ring.dma_start(out[128:N_OUT, (2 * p + 1) * TB:(2 * p + 2) * TB], o[64:104, :])

            # The cb pairs of chunk c run right before the mels of chunk
            # c+1: pairs touch only already-resident data, so when the mels
            # would stall on a fresh chunk arrival the PE fills the wait
            # with pair work instead of idling (the PE is in-order, so
            # pairs placed after stalling mels could never slide forward).
            for ci, (s, n) in enumerate(X_CHUNKS):
                if ci > 0:
                    ps_, pn = X_CHUNKS[ci - 1]
                    for kt in range(ps_, ps_ + pn):
                        cb_pair(kt, 0)
                        cb_pair(kt, 1)
                if ci < len(X_CHUNKS) - 1:
                    for kt in range(s, s + n):
                        for t in range(NTB):
                            mel(kt, t)
                else:
                    # kt49: bank-by-bank with evictions overlapping the
                    # remaining matmuls
                    for t in range(NTB):
                        mel(KT_LAST, t)
                        evict_mel(t)
            cb_pair(KT_LAST, 0)
            evict_cb(0)
            cb_pair(KT_LAST, 1)
            evict_cb(1)
    return nc


def _get_nc():
    global _NC
    if _NC is None:
        _NC = _build_nc()
        if not _NC.is_finalized():
            _NC.finalize()
    return _NC


def kernel(**inputs):
    global LAST_RESULT
    melody = np.ascontiguousarray(np.asarray(inputs["melody_tensor"], dtype=np.float32))
    lyrics = np.ascontiguousarray(np.asarray(inputs["lyrics_tensor"], dtype=np.float32))
    emb = np.asarray(inputs["emb"], dtype=np.float32)
    conv_w = np.asarray(inputs["conv_w"], dtype=np.float32)
    conv_b = np.asarray(inputs["conv_b"], dtype=np.float32)
    w_chord = np.asarray(inputs["w_chord"], dtype=np.float32)
    w_beat = np.asarray(inputs["w_beat"], dtype=np.float32)
    w_mel = np.asarray(inputs["w_mel"], dtype=np.float32)
    b_heads = np.concatenate([
        np.asarray(inputs["b_chord"], dtype=np.float32),
        np.asarray(inputs["b_beat"], dtype=np.float32),
        np.asarray(inputs["b_mel"], dtype=np.float32),
    ])
    genre = int(np.asarray(inputs["genre"]).reshape(-1)[0])
    tempo = int(np.asarray(inputs["tempo"]).reshape(-1)[0])
    key_sig = int(np.asarray(inputs["key_sig"]).reshape(-1)[0])

    # Fold conv into head weights: W'[e] = k0*W[e+1] + k1*W[e] + k2*W[e-1]
    W = np.concatenate([w_chord, w_beat, w_mel], axis=1)  # [50937, 168]
    k0, k1, k2 = (float(v) for v in conv_w[0, 0, 1, :])
    Wp = k1 * W
    Wp[:-1] += k0 * W[1:]
    Wp[1:] += k2 * W[:-1]

    # Bias: head biases + conv bias * colsum(W) + context-embedding term
    ids = [genre, 10 + tempo, 20 + key_sig, 34]
    ctx = emb[ids].sum(axis=0).astype(np.float64)  # [256]
    bias = (
        b_heads.astype(np.float64)
        + float(conv_b[0]) * W.sum(axis=0, dtype=np.float64)
        + ctx @ Wp[0:256].astype(np.float64)
    )  # [168]

    # Device operands: xT [51200, 2048] (zero padded), W' rows 256.. packed
    # [128, kt*168] with head weights per k-tile: [mel 128 | chord+beat 40]
    K_PAD = N_CORES * K_PER
    XT = np.zeros((K_PAD, T), np.float16)
    XT[0:256] = melody.T
    XT[256:K_GEMM] = lyrics.T
    Wg = np.zeros((K_PAD, N_OUT), np.float16)
    Wg[0:K_GEMM] = Wp[256:]

    in_maps = []
    for c in range(N_CORES):
        wc = (
            Wg[c * K_PER:(c + 1) * K_PER]
            .reshape(KT, 128, N_OUT)
            .transpose(1, 0, 2)
            .reshape(128, KT * N_OUT)
        )
        in_maps.append({
            "xt": XT[c * K_PER:(c + 1) * K_PER],
            "w": np.ascontiguousarray(wc),
        })

    trace = bool(os.environ.get("HARMONY_TRACE"))
    res = run_bass_kernel_spmd(_get_nc(), in_maps, core_ids=list(range(N_CORES)), trace=trace)
    LAST_RESULT = res

    acc = np.zeros((N_OUT, T), np.float64)
    for r in res.results:
        acc += r["out"].astype(np.float64)
    out = (acc + bias[:, None]).T
    return np.ascontiguousarray(out.astype(np.float32))
